# revision 75
# baseline (speedup 1.0000x reference)
"""AttentionBlock (GroupNorm + single-head attention + proj + residual) on 8 TRN2
NeuronCores.

Reference computation (B=16, C=512, H=W=32, N=H*W=1024, 32 groups):
    h   = group_norm(x, gamma, beta)                      # [B,C,H,W]
    qkv = conv1x1(h, w_qkv) + b_qkv                       # [B,3C,H,W]
    s   = q^T k / sqrt(C); a = softmax(s, axis=-1)        # [B,N,N]
    o   = v @ a^T; out = x + conv1x1(o, w_proj) + b_proj  # [B,C,H,W]

Sharding: pure data-parallel over batch. B=16 -> 2 batch elements per core,
weights replicated, no collectives.

v2 structure (per batch element, all [partition, free]):
    x         : [c, n] bf16, 4 tiles of [128, 1024] (halves the input DMA)
    h         : [c, n] fp8, per-chunk GroupNorm pipelined behind the x DMA
    g = M h   : [c, n] fp8 where M = (Wq^T Wk)*WS is host-precomputed; this
                replaces BOTH the q and k projections (s = h'Mh). The k-bias
                term is a per-i additive constant in scores and cancels in the
                softmax; the q-bias term is zero for this model (asserted on
                host, with a numpy fallback path otherwise).
    vT        : [n, c] fp8, 8 x [128, 512] via swapped matmul
    eT=exp(.) : [j, i] fp8, 8 x [128, 1024] (softmax dim on partitions)
    denom     : ones-matmul over j -> reciprocal (applied post-AV; division
                commutes with the linear map)
    av        : [c, i] fp8 = vT^T @ eT, scaled by recip
    out       : PSUM accumulates w_projT^T @ av + WS*I @ x_bf16 (the residual
                rides the matmul); single DVE drain (x 1/WS, + b_eff) -> DMA.
b_eff = w_proj @ b_v + b_proj is folded on host.

All big matmuls are fp8 DoubleRow (weights pre-scaled by WS=8); PSUM is f32.
GroupNorm statistics are computed in f32 from the bf16 x.

Emission interleaves the two batch elements so the in-order engine streams
keep TensorE fed: b1's GN stats hide under b0's g/vt phases, b1's g/vt
matmuls fill the exp/recip latency inside b0's attention.
"""

import sys

for _p in ("/opt/trn_rl_repo", "/opt/pypackages"):
    if _p not in sys.path:
        sys.path.append(_p)

import numpy as np
import ml_dtypes

import concourse.bass as bass
import concourse.bacc as bacc
import concourse.tile as tile
from concourse import mybir

# NOTE: walrus's --enable-ldw-opt=true was tried for this kernel's compile
# (every matmul pairs its own LdWeights); it hard-rejects Bass-emitted
# InstLdweights ("not compatible with LDW optimization") — keep it off.

AF = mybir.ActivationFunctionType
OP = mybir.AluOpType
F32 = mybir.dt.float32
BF16 = mybir.dt.bfloat16
FP8 = mybir.dt.float8e4
DR = mybir.MatmulPerfMode.DoubleRow
LN16 = 2.772588722239781  # eT is stored as exp(s)/16 in fp8e4 to dodge the
                          # 448 saturation point; the softmax ratio is unchanged

N_CORES = 8
B, C, H, W = 16, 512, 32, 32
N = H * W               # 1024 pixels
BPC = B // N_CORES      # batch elements per core = 2
GROUPS = 32
EPS = 1e-5
KT = C // 128           # 4 contraction chunks over channels
NT = N // 128           # 8 chunks over pixels
SCALE = 1.0 / np.sqrt(np.float32(C))
WS = 8.0          # fp8 weight pre-scale (keeps N(0,1/512) weights out of subnormals)


def build_nc():
    nc = bacc.Bacc("TRN2", target_bir_lowering=False)

    x_ext = nc.declare_dram_parameter("x", [BPC, C, N], BF16, isOutput=False)
    wm_ext = nc.declare_dram_parameter("wm", [C, C], FP8, isOutput=False)
    # wu = (w_proj @ w_v)^T: the proj matmul folded into the V projection
    # (out = Wp V A = (Wp Wv h) A since proj is linear); softmax rows sum to
    # 1 so the v/proj biases are a per-channel constant folded into x host-side
    wu_ext = nc.declare_dram_parameter("wu", [C, C], FP8, isOutput=False)
    # consts: [128, 8] f32 = gamma | beta, each [128, 4]
    consts_ext = nc.declare_dram_parameter("consts", [128, 8], F32, isOutput=False)
    # gmat: 16x16 block-diagonal of 1/16 (group-mean matrix)
    gmat_ext = nc.declare_dram_parameter("gmat", [128, 128], BF16, isOutput=False)
    ones_ext = nc.declare_dram_parameter("ones", [128, 256], FP8, isOutput=False)
    out_ext = nc.declare_dram_parameter("out", [BPC, C, N], BF16, isOutput=True)

    with tile.TileContext(nc) as tc:
        with (
            tc.tile_pool(name="wpool", bufs=1) as wpool,
            tc.tile_pool(name="xpool", bufs=2) as xpool,
            tc.tile_pool(name="hpool", bufs=2) as hpool,
            tc.tile_pool(name="gpool", bufs=1) as gpool,
            tc.tile_pool(name="vepool", bufs=1) as vepool,
            tc.tile_pool(name="avpool", bufs=1) as avpool,
            tc.tile_pool(name="opool", bufs=2) as opool,
            tc.tile_pool(name="stpool", bufs=2) as stpool,
            tc.tile_pool(name="ps_big", bufs=3, space="PSUM") as ps_big,
            tc.tile_pool(name="ps_gn", bufs=1, space="PSUM") as ps_gn,
        ):
            # ALL input DMAs are issued from the sync queue, in first-use
            # order: the scalar/ACT queue must stay clear for the exp stream
            # and DVE for stats. b0's first two chunks go as separate small
            # DMAs so GroupNorm stats can start at the earliest possible
            # moment; the rest ride as chunk-pairs.
            eps_sb = wpool.tile([128, 1], F32)
            nc.vector.memset(eps_sb, EPS)
            nln16_sb = wpool.tile([128, 1], F32)
            nc.vector.memset(nln16_sb, -LN16)

            # b0's x comes as four per-chunk tiles (separate DMAs and tiles,
            # so each GroupNorm stat starts the moment its own chunk lands —
            # tile-granular dependencies would otherwise chain every reader
            # to the LAST x write); b1's x comes as two pair tiles.
            xr = [x_ext[b].rearrange("(ko p) n -> p ko n", p=128) for b in range(BPC)]
            x0_chunks = [
                xpool.tile([128, N], BF16, name=f"x0_{ki}", bufs=1) for ki in range(KT)
            ]
            x1_pairs = [
                xpool.tile([128, 2, N], BF16, name=f"x1_{pr}", bufs=1) for pr in range(2)
            ]
            nc.sync.dma_start(out=x0_chunks[0], in_=xr[0][:, 0, :])
            nc.gpsimd.dma_start(out=x0_chunks[1], in_=xr[0][:, 1, :])
            nc.sync.dma_start(out=x0_chunks[2], in_=xr[0][:, 2, :])
            nc.gpsimd.dma_start(out=x0_chunks[3], in_=xr[0][:, 3, :])
            gmat = wpool.tile([128, 128], BF16)
            nc.sync.dma_start(out=gmat, in_=gmat_ext[:])
            consts = wpool.tile([128, 8], F32)
            nc.sync.dma_start(out=consts, in_=consts_ext[:])
            gamma_sb = consts[:, 0:4]
            beta_sb = consts[:, 4:8]
            wm = wpool.tile([128, KT, C], FP8)
            nc.sync.dma_start(out=wm, in_=wm_ext[:].rearrange("(ko p) f -> p ko f", p=128))
            wu = wpool.tile([128, KT, C], FP8)
            nc.sync.dma_start(out=wu, in_=wu_ext[:].rearrange("(ko p) f -> p ko f", p=128))
            ones = wpool.tile([128, 256], FP8)
            nc.sync.dma_start(out=ones, in_=ones_ext[:])

            def x_chunk(b, ki):
                if b == 0:
                    return x0_chunks[ki]
                return x1_pairs[ki // 2][:, ki % 2, :]



            # h as per-pair tiles: readers of pair 0 must not chain on pair 1
            h_prs = [
                [hpool.tile([128, 2, N], FP8, name=f"h_{b}_{pr}", bufs=1) for pr in range(2)]
                for b in range(BPC)
            ]
            g_sbs = [gpool.tile([128, KT, N], FP8, name="g_sb") for _ in range(BPC)]
            vTs = [vepool.tile([128, NT, C], FP8, name="vT_sb") for _ in range(BPC)]

            gn_mvs = [None, None]

            def emit_gn_stats(b):
                # per-partition mean/var via bn_stats the moment each chunk's
                # DMA lands (ACT-assisted stats for one chunk were tried and
                # regressed: the full-width scratch writes cost ACT more than
                # the parallelism saved)
                mv = stpool.tile([128, KT, 2], F32, name="mv")
                gn_mvs[b] = mv
                for ki in range(KT):
                    xc = x_chunk(b, ki)
                    stats = stpool.tile([128, 2, 6], F32, name="stats")
                    nc.vector.bn_stats(out=stats[:, 0, :], in_=xc[:, 0:512])
                    nc.vector.bn_stats(out=stats[:, 1, :], in_=xc[:, 512:1024])
                    nc.vector.bn_aggr(out=mv[:, ki, :], in_=stats)

            def emit_gn_chain(b):
                # group-reduce the 16-partition blocks with one small matmul
                # against gmat, then h = x*s + t (fp8). Split from the stats
                # so b1's Sqrt (and its ACT slot) can be queued AFTER vt(0)'s
                # drains instead of blocking them in the ACT FIFO.
                mv = gn_mvs[b]
                msq = stpool.tile([128, KT], F32, name="msq")
                nc.vector.tensor_tensor(msq, mv[:, :, 0], mv[:, :, 0], OP.mult)
                nc.vector.tensor_tensor(mv[:, :, 1], mv[:, :, 1], msq, OP.add)
                mv_bf = stpool.tile([128, KT * 2], BF16, name="mv_bf")
                nc.vector.tensor_copy(out=mv_bf, in_=mv.rearrange("p a b -> p (a b)"))
                gstat = ps_gn.tile([128, 128], F32, name="gstat", tag="gnps", bufs=1)[:, : KT * 2]
                nc.tensor.matmul(gstat, lhsT=gmat, rhs=mv_bf, start=True, stop=True)
                gs_sb = stpool.tile([128, KT * 2], F32, name="gs_sb")
                nc.vector.tensor_copy(out=gs_sb, in_=gstat)
                gmean = gs_sb[:, 0 : 2 * KT : 2]
                gex2 = gs_sb[:, 1 : 2 * KT : 2]
                gmsq = stpool.tile([128, KT], F32, name="gmsq")
                nc.vector.tensor_tensor(gmsq, gmean, gmean, OP.mult)
                gvar = stpool.tile([128, KT], F32, name="gvar")
                nc.vector.tensor_tensor(gvar, gex2, gmsq, OP.subtract)
                gstd = stpool.tile([128, KT], F32, name="gstd")
                nc.scalar.activation(out=gstd, in_=gvar, func=AF.Sqrt, bias=eps_sb)
                rstd = stpool.tile([128, KT], F32, name="rstd")
                nc.vector.reciprocal(out=rstd, in_=gstd)
                scl = stpool.tile([128, KT], F32, name="scl")
                nc.vector.tensor_tensor(scl, rstd, gamma_sb, OP.mult)
                mscl = stpool.tile([128, KT], F32, name="mscl")
                nc.vector.tensor_tensor(mscl, gmean, scl, OP.mult)
                sft = stpool.tile([128, KT], F32, name="sft")
                nc.vector.tensor_tensor(sft, beta_sb, mscl, OP.subtract)
                # b0's h: ki0-2 on DVE, ki3 on ACT (ACT is idle then; keeps
                # its scores-window queue 1.2us shorter than a 2/2 split
                # while h-pair0 still lands at the same time); b1's all ride
                # DVE so ACT stays clear for the exp streams.
                for ki in range(KT):
                    h_dst = h_prs[b][ki // 2][:, ki % 2, :]
                    if b == 0 and ki == 3:
                        nc.scalar.activation(
                            out=h_dst, in_=x_chunk(b, ki), func=AF.Identity,
                            bias=sft[:, ki : ki + 1], scale=scl[:, ki : ki + 1],
                        )
                    else:
                        nc.vector.tensor_scalar(
                            out=h_dst, in0=x_chunk(b, ki),
                            scalar1=scl[:, ki : ki + 1], scalar2=sft[:, ki : ki + 1],
                            op0=OP.mult, op1=OP.add,
                        )

            def emit_g(b):
                # g[a, j] = sum_b M[a,b] h[b,j]; ACT drain
                h_pr = h_prs[b]
                g_sb = g_sbs[b]
                for oi in range(KT):
                    ps = ps_big.tile([128, N], F32, name="mmps")
                    w_sl = wm[:, :, oi * 128 : (oi + 1) * 128]
                    for kk in range(2):
                        for ni in range(2):
                            nc.tensor.matmul(
                                ps[:, ni * 512 : (ni + 1) * 512],
                                lhsT=w_sl[:, 2 * kk : 2 * kk + 2, :],
                                rhs=h_pr[kk][:, :, ni * 512 : (ni + 1) * 512],
                                start=(kk == 0), stop=(kk == 1),
                                perf_mode=DR,
                            )
                    # b1's g drains split ACT/DVE so ACT reaches b1's exp
                    # stream sooner (b0's all-ACT: DVE is full of b1 GN then)
                    if b == 0 or oi % 2 == 0:
                        nc.scalar.activation(out=g_sb[:, oi, :], in_=ps, func=AF.Identity)
                    else:
                        nc.vector.tensor_copy(out=g_sb[:, oi, :], in_=ps)

            def emit_vt(b):
                # vT = h.T @ wu (fp8 x WS) — already carries the proj matrix
                h_pr = h_prs[b]
                vT_sb = vTs[b]
                for nn in range(NT // 2):
                    ps = ps_big.tile([128, N], F32, name="mmps")
                    for sub in range(2):
                        ni = 2 * nn + sub
                        for kk in range(2):
                            nc.tensor.matmul(
                                ps[:, sub * 512 : (sub + 1) * 512],
                                lhsT=h_pr[kk][:, :, ni * 128 : (ni + 1) * 128],
                                rhs=wu[:, 2 * kk : 2 * kk + 2, :],
                                start=(kk == 0), stop=(kk == 1),
                                perf_mode=DR,
                            )
                    # vT drains: b0 split ACT/DVE; b1 all DVE (ACT must be
                    # clear for b1's exp stream right after)
                    dst = vT_sb[:, 2 * nn : 2 * nn + 2, :].rearrange("p a b -> p (a b)")
                    if b == 0 and nn % 2 == 0:
                        nc.scalar.activation(out=dst, in_=ps, func=AF.Identity)
                    else:
                        nc.vector.tensor_copy(out=dst, in_=ps)

            eT_sbs = [None, None]
            recips = [None, None]

            def emit_scores(b):
                # eT = exp(h.T g * SCALE/WS - ln16)  [j, i]; denominator
                # matmuls interleave behind the score matmuls; recip on DVE.
                h_pr = h_prs[b]
                g_sb = g_sbs[b]
                eT_sb = vepool.tile([128, NT, N], FP8, name="eT_sb")
                eT_sbs[b] = eT_sb
                ps_d = ps_big.tile([128, N], F32, name="psden", tag="mmps")

                def denom_mm(jj):
                    for ni in range(2):
                        nc.tensor.matmul(
                            ps_d[:, ni * 512 : (ni + 1) * 512],
                            lhsT=ones.rearrange("p (two f) -> p two f", two=2),
                            rhs=eT_sb[:, 2 * jj : 2 * jj + 2, ni * 512 : (ni + 1) * 512],
                            start=(jj == 0), stop=(jj == NT // 2 - 1),
                            perf_mode=DR,
                        )

                for ji in range(NT):
                    ps = ps_big.tile([128, N], F32, name="mmps")
                    for kk in range(2):
                        for ni in range(2):
                            nc.tensor.matmul(
                                ps[:, ni * 512 : (ni + 1) * 512],
                                lhsT=g_sb[:, 2 * kk : 2 * kk + 2, ji * 128 : (ji + 1) * 128],
                                rhs=h_pr[kk][:, :, ni * 512 : (ni + 1) * 512],
                                start=(kk == 0), stop=(kk == 1),
                                perf_mode=DR,
                            )
                    nc.scalar.activation(
                        out=eT_sb[:, ji, :], in_=ps, func=AF.Exp,
                        bias=nln16_sb, scale=float(SCALE / WS),
                    )
                    # denominator for double-chunk jj interleaves two score
                    # groups later, when its exp results are already drained
                    if ji >= 3 and ji % 2 == 1:
                        denom_mm((ji - 3) // 2)
                denom_mm(NT // 2 - 1)
                recip = avpool.tile([128, N], F32, name="recip")
                recips[b] = recip
                nc.vector.reciprocal_approx_fast(out=recip, in_=ps_d)

            def emit_av(b):
                # out-tile = (vT.T @ eT) * recip + x  (vT already carries the
                # proj matrix, and b_eff rides x from the host), DoubleRow
                # over j. Two DVE ops per tile, DMA issues alternate queues.
                vT_sb = vTs[b]
                eT_sb = eT_sbs[b]
                recip = recips[b]
                for ci in range(KT):
                    ps = ps_big.tile([128, N], F32, name="mmps")
                    for jj in range(NT // 2):
                        for ni in range(2):
                            nc.tensor.matmul(
                                ps[:, ni * 512 : (ni + 1) * 512],
                                lhsT=vT_sb[:, 2 * jj : 2 * jj + 2, ci * 128 : (ci + 1) * 128],
                                rhs=eT_sb[:, 2 * jj : 2 * jj + 2, ni * 512 : (ni + 1) * 512],
                                start=(jj == 0), stop=(jj == NT // 2 - 1),
                                perf_mode=DR,
                            )
                    # both drain ops on DVE (GPSIMD's tensor ops are ~3x
                    # slower and serialize the tail); DMA issues alternate
                    # t1 in bf16: the +x op then runs at the 16-bit DVE rate
                    t1 = opool.tile([128, N], BF16, name="t1_sb")
                    o_sb = opool.tile([128, N], BF16, name="o_sb")
                    o_ext_sl = out_ext[b].rearrange("(ko p) n -> p ko n", p=128)[:, ci, :]
                    if b == 1 and ci == KT - 1:
                        # final tile: per-512 halves so the last DMA starts
                        # ~1us sooner
                        for hf in range(2):
                            sl = slice(hf * 512, (hf + 1) * 512)
                            nc.vector.tensor_tensor(t1[:, sl], ps[:, sl], recip[:, sl], OP.mult)
                            nc.vector.tensor_tensor(o_sb[:, sl], t1[:, sl], x_chunk(b, ci)[:, sl], OP.add)
                            eng = nc.gpsimd if hf == 0 else nc.sync
                            eng.dma_start(out=o_ext_sl[:, sl], in_=o_sb[:, sl])
                    else:
                        nc.vector.tensor_tensor(t1, ps, recip, OP.mult)
                        # the PSUM slot is freed by the t1 drain alone; b0's
                        # +x rides GPSIMD (slow but idle) so the DVE queue
                        # doesn't delay vt(1)'s slot turnover. b1's stays on
                        # DVE: GPSIMD's 2.5us/op would stretch the tail.
                        add_eng = nc.gpsimd if b == 0 else nc.vector
                        add_eng.tensor_tensor(o_sb, t1, x_chunk(b, ci), OP.add)
                        eng = nc.sync if b == 0 else (nc.gpsimd if ci % 2 == 0 else nc.sync)
                        eng.dma_start(out=o_ext_sl, in_=o_sb)

            emit_gn_stats(0)
            emit_gn_chain(0)
            # Gate b1's x load on b0's h completion: the compile-time list
            # scheduler orders engine FIFOs by readiness, and with b1's data
            # present early it hoists b1's 12 stat ops in front of b0's
            # critical GN chain, stalling the first matmuls by ~6us. A
            # 1-element copy from the last h tile forces the arrival order.
            for pr in range(2):
                nc.vector.tensor_copy(
                    out=x1_pairs[pr][:, 0, 0:1], in_=h_prs[0][1][:, 1, 0:1]
                )
            nc.sync.dma_start(out=x1_pairs[0], in_=xr[1][:, 0:2, :])
            nc.sync.dma_start(out=x1_pairs[1], in_=xr[1][:, 2:4, :])
            emit_g(0)
            emit_gn_stats(1)         # b1 stats overlap b0's matmul phases
            emit_vt(0)
            emit_gn_chain(1)         # b1's Sqrt queues AFTER vt(0)'s drains
            # dummy exp right after the last Sqrt: the single ACT exp-table
            # switch lands here instead of at the head of the exp stream
            # (emitting the chain before vt(0) was tried: neutral/worse)
            t1_warm = stpool.tile([128, 1], F32, name="t1_warm")
            nc.scalar.activation(out=t1_warm, in_=eps_sb, func=AF.Exp)
            emit_scores(0)
            emit_g(1)                # fills the exp/recip latency of b0
            emit_av(0)
            emit_vt(1)
            emit_scores(1)
            emit_av(1)

    nc.compile()
    return nc


_NC_CACHE = None


def _get_nc():
    global _NC_CACHE
    if _NC_CACHE is None:
        _NC_CACHE = build_nc()
    return _NC_CACHE


def _prep_consts(gamma, beta, w_qkv, b_qkv, w_proj, b_proj):
    bf = ml_dtypes.bfloat16
    f8 = ml_dtypes.float8_e4m3
    w_q, w_k, w_v = w_qkv[0:C], w_qkv[C : 2 * C], w_qkv[2 * C : 3 * C]
    b_v = b_qkv[2 * C : 3 * C]
    m = w_q.astype(np.float64).T @ w_k.astype(np.float64)  # [C, C]
    wm = np.ascontiguousarray(m.T * WS).astype(f8)         # lhsT layout [b, a]
    u = w_proj.astype(np.float64) @ w_v.astype(np.float64)  # [C, C] proj-folded V
    wu = np.ascontiguousarray(u.T * WS).astype(f8)
    b_eff = w_proj.astype(np.float64) @ b_v.astype(np.float64) + b_proj
    consts = np.stack([gamma, beta], axis=0)  # [2, 512]
    consts = consts.reshape(2, 4, 128).transpose(2, 0, 1).reshape(128, 8)
    consts = np.ascontiguousarray(consts, dtype=np.float32)
    gmat = (np.kron(np.eye(8, dtype=np.float32), np.ones((16, 16), np.float32)) / 16.0).astype(bf)
    # denominator lhsT: value WS compensates vT carrying a factor of WS
    ones = np.full((128, 256), WS, f8)
    return wm, wu, b_eff.astype(np.float32), consts, gmat, ones


def make_in_maps(x, gamma, beta, w_qkv, b_qkv, w_proj, b_proj):
    bf = ml_dtypes.bfloat16
    x = np.asarray(x, np.float32)
    gamma = np.asarray(gamma, np.float32)
    beta = np.asarray(beta, np.float32)
    w_qkv = np.asarray(w_qkv, np.float32)
    b_qkv = np.asarray(b_qkv, np.float32)
    w_proj = np.asarray(w_proj, np.float32)
    b_proj = np.asarray(b_proj, np.float32)
    wm, wu, b_eff, consts, gmat, ones = _prep_consts(
        gamma, beta, w_qkv, b_qkv, w_proj, b_proj
    )
    # b_eff rides the residual input: GroupNorm is invariant to a per-channel
    # shift (the mean absorbs it), and softmax rows sum to 1, so shipping
    # x + b_eff makes out = (x + b_eff) + U h A exactly the reference result.
    xr = np.ascontiguousarray(
        (x.reshape(B, C, N) + b_eff[None, :, None]).astype(bf)
    )
    return [
        {
            "x": xr[i * BPC : (i + 1) * BPC],
            "wm": wm,
            "wu": wu,
            "consts": consts,
            "gmat": gmat,
            "ones": ones,
        }
        for i in range(N_CORES)
    ]


def _numpy_fallback(x, gamma, beta, w_qkv, b_qkv, w_proj, b_proj):
    # Exact reference implementation; only used when b_q is nonzero (the
    # device graph folds Wq^T Wk and drops the q-bias term, which is exact
    # for this model where b_qkv == 0).
    Bs, Cs, Hs, Ws_ = x.shape
    g = x.reshape(Bs, GROUPS, Cs // GROUPS, Hs, Ws_)
    mu = g.mean(axis=(2, 3, 4), keepdims=True)
    var = g.var(axis=(2, 3, 4), keepdims=True)
    g = (g - mu) / np.sqrt(var + EPS)
    h = g.reshape(Bs, Cs, Hs, Ws_) * gamma[None, :, None, None] + beta[None, :, None, None]
    hn = h.reshape(Bs, Cs, N)
    qkv = np.einsum("bcn,oc->bon", hn, w_qkv) + b_qkv[None, :, None]
    q, k, v = qkv[:, :Cs], qkv[:, Cs : 2 * Cs], qkv[:, 2 * Cs :]
    s = np.einsum("bci,bcj->bij", q, k) / np.sqrt(np.float32(Cs))
    s = s - s.max(axis=-1, keepdims=True)
    e = np.exp(s)
    a = e / e.sum(axis=-1, keepdims=True)
    o = np.einsum("bij,bcj->bci", a, v)
    o = np.einsum("bcn,oc->bon", o, w_proj) + b_proj[None, :, None]
    return (x + o.reshape(Bs, Cs, Hs, Ws_)).astype(np.float32)


def kernel(x, gamma, beta, w_qkv, b_qkv, w_proj, b_proj):
    from concourse.bass_utils import run_bass_kernel_spmd

    x = np.asarray(x, np.float32)
    b_qkv = np.asarray(b_qkv, np.float32)
    if np.abs(b_qkv[0:C]).max() > 1e-7:
        return _numpy_fallback(
            x, np.asarray(gamma, np.float32), np.asarray(beta, np.float32),
            np.asarray(w_qkv, np.float32), b_qkv,
            np.asarray(w_proj, np.float32), np.asarray(b_proj, np.float32),
        )

    nc = _get_nc()
    in_maps = make_in_maps(x, gamma, beta, w_qkv, b_qkv, w_proj, b_proj)
    res = run_bass_kernel_spmd(nc, in_maps, core_ids=list(range(N_CORES)))
    out = np.concatenate([res.results[i]["out"] for i in range(N_CORES)], axis=0)
    return np.ascontiguousarray(out.reshape(B, C, H, W), dtype=np.float32)
